# revision 37
# baseline (speedup 1.0000x reference)
"""Trainium2 Bass kernel for nn_LiveNet (2-layer MLP: relu(x@W1+b1)@W2+b2).

Sharding: pure data-parallel over batch across 8 NeuronCores (no
collectives).  Each core computes y_i = relu(x_i @ W1 + b1) @ W2 + b2 for
its 512-row batch shard.

All matmul operands are fp16 (host-side cast): same 1 cycle/row PE speed
as fp32r but half the HBM traffic, which takes the DMA stream off the
360 GB/s ridge (fp32 needed 38.5 MB/core vs ~55 us of per-GEMM compute;
fp16 needs 19.5 MB).  PSUM accumulation is fp32; biases fp32; output fp32.

Per-core dataflow:
  Warmup: matmuls on a zeroed junk tile keep the PE busy from t~=0.1us so
          the 3us p-state ramp (0.65/1.2 GHz -> 2.4 GHz) burns off while
          the first operand DMAs (~2us of DGE+transfer+sem latency) are
          still in flight; more junk fills m=0's DMA-cadence gaps.
  Queues: DMA instructions hold their issuing engine's sequencer for the
          whole transfer, so they are spread over two hardware queues --
          SP carries xt/b1/w2/b2/y, ACT carries the W1 chunk stream --
          while the transfers share the 360GB/s DMA fleet.  GEMM1 evicts
          run on DVE (relu(ps+b1) as one tensor_scalar) so they never
          head-of-line block the ACT weight stream.
  GEMM1:  hT[hid, batch] tiles: lhsT = W1[k-tile, m-chunk] (stationary),
          rhs = xT[k-tile, :] (moving, N=512).  PSUM accumulates over the
          8 k-tiles, then DVE applies bias+ReLU evicting to SBUF fp16.
          W2 k-slices stream into a resident SBUF tile in the same loop
          (one 256KB slice per m-iter, paced behind each W1 chunk load).
  GEMM2:  chunk-serial: for each of 8 output chunks (4 batch x 2 out-col),
          32 k-matmuls accumulate in one PSUM bank, then DVE adds b2 and
          the 256KB y chunk DMAs out while the next chunk computes.  W2 is
          already resident, so GEMM2 has no DMA dependencies; the final
          chunk is split 256/128/64/64 cols so the after-last-matmul tail
          is one small evict + store (~2.6us instead of ~7.5us).
"""

import os
import sys

import numpy as np

for _p in ("/opt/trn_rl_repo", "/root/.axon_site/_ro/trn_rl_repo"):
    if os.path.isdir(_p) and _p not in sys.path:
        sys.path.append(_p)

import concourse.bacc as bacc
import concourse.bass as bass
import concourse.tile as tile
from concourse import mybir
from concourse.bass_utils import run_bass_kernel_spmd

N_CORES = 8
B, N_IN, N_HID, N_OUT = 4096, 1024, 4096, 1024
BSH = B // N_CORES          # 512 batch rows per core
P = 128                     # SBUF partitions
KT1 = N_IN // P             # 8  k-tiles in GEMM1
MT1 = N_HID // P            # 32 m-tiles (hid) in GEMM1
KT2 = N_HID // P            # 32 k-tiles in GEMM2
MT2 = BSH // P              # 4  batch m-tiles in GEMM2
NCH = 512                   # moving free dim per matmul
NT2 = N_OUT // NCH          # 2  out chunks in GEMM2
N_WARM = 8                 # junk warmup matmuls (128 rows each)
N_FILL = 0                  # junk fill matmuls between m=0 k-steps

F32 = mybir.dt.float32
F16 = mybir.dt.float16


def build_nc(reps=1):
    nc = bacc.Bacc("TRN2", target_bir_lowering=False, debug=False,
                   num_devices=N_CORES)

    xt = nc.declare_dram_parameter("xt", [N_IN, BSH], F16, isOutput=False)
    w1r = nc.declare_dram_parameter("w1r", [MT1, P, N_IN], F16, isOutput=False)
    w2r = nc.declare_dram_parameter("w2r", [KT2, P, N_OUT], F16, isOutput=False)
    b1t = nc.declare_dram_parameter("b1t", [P, MT1], F32, isOutput=False)
    b2r = nc.declare_dram_parameter("b2r", [P, N_OUT], F32, isOutput=False)
    y = nc.declare_dram_parameter("y", [BSH, N_OUT], F32, isOutput=True)

    with tile.TileContext(nc) as tc:
        with (
            tc.tile_pool(name="const", bufs=1) as const,
            tc.tile_pool(name="xt", bufs=1) as xt_pool,
            tc.tile_pool(name="ht", bufs=1) as ht_pool,
            tc.tile_pool(name="w1", bufs=4) as w1_pool,
            tc.tile_pool(name="w2", bufs=1) as w2_pool,
            tc.tile_pool(name="yout", bufs=4) as y_pool,
            tc.tile_pool(name="ps", bufs=8, space=bass.MemorySpace.PSUM) as ps_pool,
        ):
            # PE warmup on a zeroed tile: starts as soon as the memset
            # lands (~0.1us), long before the first operand DMA completes,
            # so the p-state ramp overlaps the startup DMA latency.
            junk = const.tile([P, P], F16)
            nc.gpsimd.memset(junk[:], 0.0)
            ps_junk = ps_pool.tile([P, P], F32, tag="ps", name="ps_junk")
            for j in range(N_WARM):
                nc.tensor.matmul(
                    ps_junk[:], junk[:], junk[:],
                    start=(j == 0), stop=(j == N_WARM - 1),
                )

            # Two DMA queues: SP carries xt/b1/w2/b2/y, ACT carries the
            # W1 chunk stream (plus xt 1/3/7).  Each queue's per-DMA
            # sequencer hold overlaps the other's; transfers still share
            # the 360GB/s DMA fleet.  Startup order mirrors first-use
            # order.
            xt_sb = [
                xt_pool.tile([P, BSH], F16, tag=f"xtk_{k}", name=f"xtk_{k}")
                for k in range(KT1)
            ]
            w1_tiles = {}

            def w1_load(m):
                t = w1_pool.tile([P, N_IN], F16, tag="w1", name="w1_sb")
                nc.scalar.dma_start(out=t[:], in_=w1r[m])
                w1_tiles[m] = t

            nc.sync.dma_start(out=xt_sb[0][:], in_=xt[0:P, :])
            w1_load(0)
            nc.scalar.dma_start(out=xt_sb[1][:], in_=xt[P:2 * P, :])
            nc.scalar.dma_start(out=xt_sb[3][:], in_=xt[3 * P:4 * P, :])
            for k in (2, 4, 5, 6):
                nc.sync.dma_start(out=xt_sb[k][:], in_=xt[k * P:(k + 1) * P, :])
            nc.scalar.dma_start(out=xt_sb[7][:], in_=xt[7 * P:8 * P, :])
            b1_sb = const.tile([P, MT1], F32)
            nc.sync.dma_start(out=b1_sb[:], in_=b1t[:])
            # w1_1 arrives as two half-chunks: its full arrival anchors the
            # dense PE stream, but m=1's k=0..3 only need the first half,
            # so the split moves the anchor ~360ns earlier.
            w1_1h = []
            for h in range(2):
                t = w1_pool.tile([P, N_IN // 2], F16, tag="w1h",
                                 name=f"w1_1h{h}")
                nc.scalar.dma_start(
                    out=t[:], in_=w1r[1, :, h * (N_IN // 2):(h + 1) * (N_IN // 2)]
                )
                w1_1h.append(t)
            w1_tiles[1] = tuple(w1_1h)
            w1_load(2)
            # Prime DVE with the b1-load DMA wait so the evicts (which
            # already wait on the PE sem) don't exceed the per-instruction
            # sync-wait budget in walrus codegen.
            prime1 = const.tile([P, 1], F32)
            nc.vector.tensor_copy(prime1[:], b1_sb[:, 0:1])

            b2_sb = const.tile([P, N_OUT], F32)

            for rep in range(reps):
                if rep > 0:
                    for mm in (0, 1, 2):
                        w1_load(mm)
                # hT resident: [128, 32, 512]; tile j = hid rows j*128..+127.
                ht_sb = ht_pool.tile([P, MT1, BSH], F16, tag="ht",
                                     name="ht_sb")
                # W2 resident: [128, 32, 1024]; slice k = hid rows k*128..+127,
                # streamed in during GEMM1.
                w2_sb = w2_pool.tile([P, KT2, N_OUT], F16, tag="w2",
                                     name="w2_sb")

                # ---- GEMM1: hT = relu(W1.T-tiled @ xT + b1) ----
                for m in range(MT1):
                    # Prefetch this rep's W1 chunk m+3 (2 iterations ahead)
                    # and pace the W2 stream at one k-slice per m-iter
                    # (1456ns of DMA per 1707ns of compute), so neither
                    # front-runs operands GEMM1 needs imminently.
                    mp = m + 3
                    if mp < MT1:
                        w1_load(mp)
                    if rep == 0 and m >= 2:
                        nc.sync.dma_start(out=w2_sb[:, m - 2, :],
                                          in_=w2r[m - 2])
                    ps = ps_pool.tile([P, BSH], F32, tag="ps", name="ps")
                    w1_sb = w1_tiles.pop(m)
                    halved = isinstance(w1_sb, tuple)
                    for k in range(KT1):
                        if halved:
                            lhs = w1_sb[k // 4][:, (k % 4) * P:(k % 4 + 1) * P]
                        else:
                            lhs = w1_sb[:, k * P:(k + 1) * P]
                        nc.tensor.matmul(
                            ps[:],
                            lhs,
                            xt_sb[k][:],
                            start=(k == 0),
                            stop=(k == KT1 - 1),
                        )
                        if rep == 0 and m == 0 and k < KT1 - 1:
                            # m=0 is DMA-cadence-bound (each k-matmul waits
                            # ~300ns for its xt tile); keep the PE warm on
                            # junk between them instead of idling.
                            for _ in range(N_FILL):
                                nc.tensor.matmul(
                                    ps_junk[:], junk[:], junk[:],
                                    start=True, stop=True,
                                    skip_group_check=True,
                                )
                    # relu(ps + b1) on DVE, keeping the ACT queue free for
                    # the W1 chunk stream.
                    nc.vector.tensor_scalar(
                        ht_sb[:, m, :], ps[:], b1_sb[:, m:m + 1], 0.0,
                        mybir.AluOpType.add, mybir.AluOpType.max,
                    )

                # Last two W2 slices land during GEMM2's first chunk (its
                # ascending k-loop touches k=30,31 only near the chunk end).
                if rep == 0:
                    nc.sync.dma_start(out=w2_sb[:, 30, :], in_=w2r[30])
                    nc.sync.dma_start(out=w2_sb[:, 31, :], in_=w2r[31])
                    nc.sync.dma_start(out=b2_sb[:], in_=b2r[:])
                    # Prime DVE with the b2-load DMA wait (same sync-wait
                    # budget workaround as prime1).
                    prime2 = const.tile([P, 1], F32)
                    nc.vector.tensor_copy(prime2[:], b2_sb[:, 0:1])

                # ---- GEMM2: y = hT.T @ W2 + b2, chunk-serial ----
                # The final chunk is split 256/128/64/64 so the
                # after-last-matmul tail is an eighth-size evict + store.
                chunks = []
                for n in range(NT2):
                    for m in range(MT2):
                        if n == NT2 - 1 and m == MT2 - 1:
                            for c0, cw in ((0, 256), (256, 128), (384, 64),
                                           (448, 64)):
                                chunks.append((m, n * NCH + c0, cw))
                        else:
                            chunks.append((m, n * NCH, NCH))
                for m, c0, cw in chunks:
                    ps2 = ps_pool.tile([P, cw], F32, tag="ps", name="ps2")
                    for k in range(KT2):
                        nc.tensor.matmul(
                            ps2[:],
                            ht_sb[:, k, m * P:(m + 1) * P],
                            w2_sb[:, k, c0:c0 + cw],
                            start=(k == 0),
                            stop=(k == KT2 - 1),
                        )
                    y_sb = y_pool.tile([P, cw], F32, tag="y", name="y_sb")
                    nc.vector.tensor_add(
                        y_sb[:], ps2[:], b2_sb[:, c0:c0 + cw],
                    )
                    nc.sync.dma_start(
                        out=y[m * P:(m + 1) * P, c0:c0 + cw],
                        in_=y_sb[:],
                    )
    nc.compile()
    return nc


def _prep_shared(W1, b1, W2, b2):
    W1 = np.asarray(W1, dtype=np.float32)
    # w1r[m, p, k*128+c] = W1[k*128+p, m*128+c]
    w1r = np.ascontiguousarray(
        W1.reshape(KT1, P, MT1, P).transpose(2, 1, 0, 3)
    ).reshape(MT1, P, N_IN).astype(np.float16)
    b1t = np.ascontiguousarray(
        np.asarray(b1, dtype=np.float32).reshape(MT1, P).T
    )
    b2r = np.ascontiguousarray(
        np.broadcast_to(np.asarray(b2, dtype=np.float32), (P, N_OUT))
    )
    # w2r[k, p, :] = W2[k*128+p, :]
    w2r = np.asarray(W2, dtype=np.float32).reshape(KT2, P, N_OUT).astype(
        np.float16
    )
    return w1r, b1t, w2r, b2r


def kernel(x, W1, b1, W2, b2):
    x = np.asarray(x, dtype=np.float32)
    w1r, b1t, w2r, b2r = _prep_shared(W1, b1, W2, b2)

    in_maps = []
    for i in range(N_CORES):
        xt_i = np.ascontiguousarray(
            x[i * BSH:(i + 1) * BSH, :].T.astype(np.float16)
        )
        in_maps.append(
            {"xt": xt_i, "w1r": w1r, "w2r": w2r, "b1t": b1t, "b2r": b2r}
        )

    nc = build_nc()
    res = run_bass_kernel_spmd(nc, in_maps, list(range(N_CORES)))
    y = np.concatenate(
        [np.asarray(res.results[i]["y"]) for i in range(N_CORES)], axis=0
    )
    return y.astype(np.float32)


if __name__ == "__main__":
    rng = np.random.default_rng(0)
    x = rng.standard_normal((B, N_IN), dtype=np.float32)
    W1 = rng.standard_normal((N_IN, N_HID), dtype=np.float32) / 32
    b1 = rng.standard_normal((N_HID,), dtype=np.float32) / 32
    W2 = rng.standard_normal((N_HID, N_OUT), dtype=np.float32) / 64
    b2 = rng.standard_normal((N_OUT,), dtype=np.float32) / 64
    y = kernel(x, W1, b1, W2, b2)
    h = np.maximum(x @ W1 + b1, 0)
    y_ref = h @ W2 + b2
    err = np.linalg.norm(y - y_ref) / np.linalg.norm(y_ref)
    print("rel_l2:", err)
